# revision 6
# baseline (speedup 1.0000x reference)
"""Trainium2 Bass kernel for nn_CMIAttentionMatrixForAcrobot.

Reference computation (all fp32):
    q     = data_q @ W_q.T + b_q                  # [4096, 4096]
    new_q = q.T @ W_lin.T + b_lin                 # [4096, 6]
    k     = data_k @ W_k.T + b_k                  # [6, 4096]
    ctx   = new_q.T                               # [6, 4096]
    k_mod = relu6(k^2 + 2k + ctx*(1+|k|))         # [6, 4096]
    out   = (q @ k_mod.T) / 64                    # [4096, 6]

Factorization used here:
  - ctx = (W_lin @ data_q) @ W_q.T + rowsum(W_lin) x b_q + b_lin  (associativity),
    so k_mod is computable from ~0.6 GFLOP of tiny [6,.] host BLAS.
  - q @ k_mod.T = (data_q @ W_q.T) @ k_mod.T + ones x (k_mod @ b_q),
    so the device never needs the bias and runs two pure chained matmuls.

Device work (tensor-parallel over 8 cores, W_q rows sharded -> q column-sharded):
  per core s (columns js = 512s..512s+512):
    qT_s   [512, 4096] = W_q[js].T^T-chunks @ data_q.T    (1024 matmuls, N=512)
    dotT_s [6, 4096]   = k_mod[:, js].T^T-chunks @ qT_s   (32 matmuls)
  Host sums the 8 dotT_s partials (contraction over msg_dim is sharded), adds
  the bias row k_mod @ b_q, transposes, scales by 1/64.

All matmuls run as float32r (fp32 bits, full-rate reduced-precision PE mode).
"""

import numpy as np

P = 128
MSG = 4096          # msg_dim
DIN = 4096          # data_q inner dim / row count
N_CORES = 8
JS = MSG // N_CORES  # 512 columns of q per core

_NC_CACHE = {}


def round_f32r(a):
    """Round fp32 array to the float32r representation: ieee fp32 with the
    mantissa rounded (nearest-even) to 11 bits, low 12 bits zero."""
    u = np.ascontiguousarray(a, dtype=np.float32).view(np.uint32)
    lsb = (u >> np.uint32(12)) & np.uint32(1)
    r = (u + np.uint32(0x7FF) + lsb) & np.uint32(0xFFFFF000)
    return r.view(np.float32)


def build_nc(din=DIN, js=JS, n_free=512, dtype_name="float32r"):
    """Build the per-core Bass module.

    Inputs (per core):
      dqT [din, din]        data_q.T (replicated across cores)
      wqT [128, din/128, js]  W_q[js_rows].T prearranged as [p, d_chunk, j]
      kmT [128, js/128, 6]    k_mod[:, js_cols].T prearranged as [p, j_chunk, c]
    Output:
      dotT [6, din]         partial (data_q @ W_q_s.T) @ k_mod_s.T, transposed
    """
    import concourse.mybir as mybir
    import concourse.tile as tile
    from concourse import bacc

    DC = din // P            # d chunks (contraction of matmul 1)
    JC = js // P             # j chunks (contraction of matmul 2)
    NT = din // n_free       # output column tiles
    DG = 8 if DC % 8 == 0 else DC  # d-chunks per DMA group
    NDG = DC // DG

    mm_dt = getattr(mybir.dt, dtype_name)

    nc = bacc.Bacc(
        "TRN2", target_bir_lowering=False, debug=False, enable_partition_id=False
    )
    dqT = nc.dram_tensor("dqT", [din, din], mm_dt, kind="ExternalInput").ap()
    wqT = nc.dram_tensor("wqT", [P, DC, js], mm_dt, kind="ExternalInput").ap()
    kmT = nc.dram_tensor("kmT", [P, JC, 6], mm_dt, kind="ExternalInput").ap()
    dotT = nc.dram_tensor("dotT", [6, din], mybir.dt.float32, kind="ExternalOutput").ap()

    # view of dqT with the partition (d) dim split out: [p, d_outer, n]
    dqT_t = dqT.rearrange("(o p) n -> p o n", p=P)

    with tile.TileContext(nc) as tc:
        with (
            tc.tile_pool(name="const", bufs=1) as const,
            tc.tile_pool(name="dqp", bufs=3) as dqp,
            tc.tile_pool(name="qtp", bufs=2) as qtp,
            tc.tile_pool(name="outp", bufs=2) as outp,
            tc.tile_pool(name="ps1", bufs=6, space="PSUM") as ps1,
            tc.tile_pool(name="ps2", bufs=2, space="PSUM") as ps2,
        ):
            # resident weights: W_q shard, split into NDG groups so early
            # matmuls can start before the whole 8.4 MB load finishes
            wq_sb = []
            for g in range(NDG):
                t = const.tile([P, DG, js], mm_dt, name=f"wq{g}")
                nc.sync.dma_start(t[:], wqT[:, g * DG:(g + 1) * DG, :])
                wq_sb.append(t)
            km_sb = const.tile([P, JC, 6], mm_dt, name="km")
            nc.sync.dma_start(km_sb[:], kmT[:])

            for nt in range(NT):
                n_lo = nt * n_free
                # matmul 1: psum[j][:, :] += wq[:, d, j*128:+128].T @ dq[:, d, :]
                psums = [
                    ps1.tile([P, n_free], mybir.dt.float32, name="ps1t", tag="ps1t")
                    for _ in range(JC)
                ]
                for g in range(NDG):
                    dq_t = dqp.tile([P, DG, n_free], mm_dt, name="dqt", tag="dqt")
                    nc.sync.dma_start(
                        dq_t[:], dqT_t[:, g * DG:(g + 1) * DG, n_lo:n_lo + n_free]
                    )
                    for d in range(DG):
                        first = g == 0 and d == 0
                        last = g == NDG - 1 and d == DG - 1
                        for j in range(JC):
                            nc.tensor.matmul(
                                psums[j][:],
                                wq_sb[g][:, d, j * P:(j + 1) * P],
                                dq_t[:, d, :],
                                start=first,
                                stop=last,
                            )
                # evict qT tiles to SBUF
                qt = qtp.tile([P, JC, n_free], mm_dt, name="qt", tag="qt")
                for j in range(JC):
                    nc.vector.tensor_copy(qt[:, j, :], psums[j][:])

                # matmul 2: dotT[:, ntile] = sum_j km[:, j, :].T @ qT[:, j, :]
                pd = ps2.tile([6, n_free], mybir.dt.float32, name="pd", tag="pd")
                for j in range(JC):
                    nc.tensor.matmul(
                        pd[:],
                        km_sb[:, j, :],
                        qt[:, j, :],
                        start=(j == 0),
                        stop=(j == JC - 1),
                    )
                ot = outp.tile([6, n_free], mybir.dt.float32, name="ot", tag="ot")
                nc.vector.tensor_copy(ot[:], pd[:])
                nc.sync.dma_start(dotT[:, n_lo:n_lo + n_free], ot[:])
    nc.compile()
    return nc


def host_prep(inputs, n_cores=N_CORES):
    """Host-side small algebra + per-core input prearrangement."""
    dq = np.ascontiguousarray(np.asarray(inputs["data_q"], dtype=np.float32))
    dk = np.asarray(inputs["data_k"], dtype=np.float32)
    Wq = np.asarray(inputs["W_q"], dtype=np.float32)
    bq = np.asarray(inputs["b_q"], dtype=np.float32)
    Wlin = np.asarray(inputs["W_lin"], dtype=np.float32)
    blin = np.asarray(inputs["b_lin"], dtype=np.float32)
    Wk = np.asarray(inputs["W_k"], dtype=np.float32)
    bk = np.asarray(inputs["b_k"], dtype=np.float32)

    f8 = np.float64
    T = Wlin.astype(f8) @ dq.astype(f8)                     # [6, din]
    ctx = (
        T @ Wq.astype(f8).T
        + Wlin.astype(f8).sum(1)[:, None] * bq.astype(f8)[None, :]
        + blin.astype(f8)[:, None]
    )                                                       # [6, msg]
    k = dk.astype(f8) @ Wk.astype(f8).T + bk.astype(f8)[None, :]
    kmod = np.clip(k * k + 2.0 * k + ctx * (1.0 + np.abs(k)), 0.0, 6.0)
    bias_row = kmod @ bq.astype(f8)                         # [6]

    dqT = round_f32r(dq.T)
    kmod32 = kmod.astype(np.float32)

    js = Wq.shape[0] // n_cores
    in_maps = []
    for s in range(n_cores):
        Wq_s = Wq[s * js:(s + 1) * js, :]                  # [js, din]
        # wqT[p, o, j] = Wq_s[j, o*128+p]
        wq_pre = round_f32r(
            Wq_s.reshape(js, -1, P).transpose(2, 1, 0)
        )                                                  # [128, din/128, js]
        km_s = kmod32[:, s * js:(s + 1) * js]              # [6, js]
        # kmT[p, jc, c] = km_s[c, jc*128+p]
        km_pre = round_f32r(
            km_s.T.reshape(-1, P, 6).transpose(1, 0, 2)
        )                                                  # [128, js/128, 6]
        in_maps.append({"dqT": dqT, "wqT": wq_pre, "kmT": km_pre})
    return in_maps, bias_row


def host_finish(partials, bias_row):
    dotT = np.zeros_like(partials[0], dtype=np.float64)
    for p in partials:
        dotT += p
    return ((dotT.T + bias_row[None, :]) / 64.0).astype(np.float32)


def kernel(**inputs):
    from concourse.bass_utils import run_bass_kernel_spmd

    if "nc" not in _NC_CACHE:
        _NC_CACHE["nc"] = build_nc()
    nc = _NC_CACHE["nc"]

    in_maps, bias_row = host_prep(inputs)
    res = run_bass_kernel_spmd(nc, in_maps, core_ids=list(range(N_CORES)))
    partials = [r["dotT"] for r in res.results]
    return host_finish(partials, bias_row)


# revision 9
# speedup vs baseline: 1.0146x; 1.0146x over previous
"""Trainium2 Bass kernel for nn_CMIAttentionMatrixForAcrobot.

Reference computation (all fp32):
    q     = data_q @ W_q.T + b_q                  # [4096, 4096]
    new_q = q.T @ W_lin.T + b_lin                 # [4096, 6]
    k     = data_k @ W_k.T + b_k                  # [6, 4096]
    ctx   = new_q.T                               # [6, 4096]
    k_mod = relu6(k^2 + 2k + ctx*(1+|k|))         # [6, 4096]
    out   = (q @ k_mod.T) / 64                    # [4096, 6]

Factorization used here:
  - ctx = (W_lin @ data_q) @ W_q.T + rowsum(W_lin) x b_q + b_lin  (associativity),
    so k_mod is computable from ~0.6 GFLOP of tiny [6,.] host BLAS.
  - q @ k_mod.T = (data_q @ W_q.T) @ k_mod.T + ones x (k_mod @ b_q),
    so the device never needs the bias and runs two pure chained matmuls.

Device work (tensor-parallel over 8 cores, W_q rows sharded -> q column-sharded):
  per core s (columns js = 512s..512s+512):
    qT_s   [512, 4096] = W_q[js].T^T-chunks @ data_q.T    (1024 matmuls, N=512)
    dotT_s [6, 4096]   = k_mod[:, js].T^T-chunks @ qT_s   (32 matmuls)
  Host sums the 8 dotT_s partials (contraction over msg_dim is sharded), adds
  the bias row k_mod @ b_q, transposes, scales by 1/64.

Matmul dtype: float16 (full-rate on the PE; 11-bit mantissa). Measured HW
alternatives: float32r is ~6x slower than the cost model claims; bf16 is the
same speed but ~4x less precise.
"""

import numpy as np

P = 128
MSG = 4096          # msg_dim
DIN = 4096          # data_q inner dim / row count
N_CORES = 8
JS = MSG // N_CORES  # 512 columns of q per core
DTYPE_NAME = "float16"
NP_DT = np.float16

_NC_CACHE = {}


def round_f32r(a):
    """Round fp32 array to the float32r representation: ieee fp32 with the
    mantissa rounded (nearest-even) to 11 bits, low 12 bits zero."""
    u = np.ascontiguousarray(a, dtype=np.float32).view(np.uint32)
    lsb = (u >> np.uint32(12)) & np.uint32(1)
    r = (u + np.uint32(0x7FF) + lsb) & np.uint32(0xFFFFF000)
    return r.view(np.float32)


def build_nc(din=DIN, js=JS, n_free=512, dtype_name=DTYPE_NAME, repeat=1):
    """Build the per-core Bass module.

    Inputs (per core):
      dqT [din, din]        data_q.T (replicated across cores)
      wqT [128, din/128, js]  W_q[js_rows].T prearranged as [p, d_chunk, j]
      kmT [128, js/128, 6]    k_mod[:, js_cols].T prearranged as [p, j_chunk, c]
    Output:
      dotT [6, din]         partial (data_q @ W_q_s.T) @ k_mod_s.T, transposed
    """
    import concourse.mybir as mybir
    import concourse.tile as tile
    from concourse import bacc

    DC = din // P            # d chunks (contraction of matmul 1)
    JC = js // P             # j chunks (contraction of matmul 2)
    NT = din // n_free       # output column tiles
    DG = 8 if DC % 8 == 0 else DC  # d-chunks per DMA group
    NDG = DC // DG

    mm_dt = getattr(mybir.dt, dtype_name)

    nc = bacc.Bacc(
        "TRN2", target_bir_lowering=False, debug=False, enable_partition_id=False
    )
    dqT = nc.dram_tensor("dqT", [din, din], mm_dt, kind="ExternalInput").ap()
    wqT = nc.dram_tensor("wqT", [P, DC, js], mm_dt, kind="ExternalInput").ap()
    kmT = nc.dram_tensor("kmT", [P, JC, 6], mm_dt, kind="ExternalInput").ap()
    dotT = nc.dram_tensor("dotT", [6, din], mybir.dt.float32, kind="ExternalOutput").ap()

    # view of dqT with the partition (d) dim split out: [p, d_outer, n]
    dqT_t = dqT.rearrange("(o p) n -> p o n", p=P)

    with tile.TileContext(nc) as tc:
        with (
            tc.tile_pool(name="const", bufs=1) as const,
            tc.tile_pool(name="dqp", bufs=3) as dqp,
            tc.tile_pool(name="qtp", bufs=2) as qtp,
            tc.tile_pool(name="outp", bufs=2) as outp,
            tc.tile_pool(name="ps1", bufs=6, space="PSUM") as ps1,
            tc.tile_pool(name="ps2", bufs=2, space="PSUM") as ps2,
        ):
            # resident weights: W_q shard, split into NDG groups so early
            # matmuls can start before the whole 8.4 MB load finishes
            wq_sb = []
            for g in range(NDG):
                t = const.tile([P, DG, js], mm_dt, name=f"wq{g}")
                nc.sync.dma_start(t[:], wqT[:, g * DG:(g + 1) * DG, :])
                wq_sb.append(t)
            km_sb = const.tile([P, JC, 6], mm_dt, name="km")
            nc.sync.dma_start(km_sb[:], kmT[:])

            for nt_rep in range(NT * repeat):
                nt = nt_rep % NT
                n_lo = nt * n_free
                # matmul 1: psum[j][:, :] += wq[:, d, j*128:+128].T @ dq[:, d, :]
                psums = [
                    ps1.tile([P, n_free], mybir.dt.float32, name="ps1t", tag="ps1t")
                    for _ in range(JC)
                ]
                for g in range(NDG):
                    dq_t = dqp.tile([P, DG, n_free], mm_dt, name="dqt", tag="dqt")
                    nc.sync.dma_start(
                        dq_t[:], dqT_t[:, g * DG:(g + 1) * DG, n_lo:n_lo + n_free]
                    )
                    for d in range(DG):
                        first = g == 0 and d == 0
                        last = g == NDG - 1 and d == DG - 1
                        for j in range(JC):
                            nc.tensor.matmul(
                                psums[j][:],
                                wq_sb[g][:, d, j * P:(j + 1) * P],
                                dq_t[:, d, :],
                                start=first,
                                stop=last,
                            )
                # evict qT tiles to SBUF
                qt = qtp.tile([P, JC, n_free], mm_dt, name="qt", tag="qt")
                for j in range(JC):
                    nc.vector.tensor_copy(qt[:, j, :], psums[j][:])

                # matmul 2: dotT[:, ntile] = sum_j km[:, j, :].T @ qT[:, j, :]
                pd = ps2.tile([6, n_free], mybir.dt.float32, name="pd", tag="pd")
                for j in range(JC):
                    nc.tensor.matmul(
                        pd[:],
                        km_sb[:, j, :],
                        qt[:, j, :],
                        start=(j == 0),
                        stop=(j == JC - 1),
                    )
                ot = outp.tile([6, n_free], mybir.dt.float32, name="ot", tag="ot")
                nc.vector.tensor_copy(ot[:], pd[:])
                nc.sync.dma_start(dotT[:, n_lo:n_lo + n_free], ot[:])
    nc.compile()
    return nc


def host_prep(inputs, n_cores=N_CORES):
    """Host-side small algebra + per-core input prearrangement."""
    dq = np.ascontiguousarray(np.asarray(inputs["data_q"], dtype=np.float32))
    dk = np.asarray(inputs["data_k"], dtype=np.float32)
    Wq = np.asarray(inputs["W_q"], dtype=np.float32)
    bq = np.asarray(inputs["b_q"], dtype=np.float32)
    Wlin = np.asarray(inputs["W_lin"], dtype=np.float32)
    blin = np.asarray(inputs["b_lin"], dtype=np.float32)
    Wk = np.asarray(inputs["W_k"], dtype=np.float32)
    bk = np.asarray(inputs["b_k"], dtype=np.float32)

    f8 = np.float64
    T = Wlin.astype(f8) @ dq.astype(f8)                     # [6, din]
    ctx = (
        T @ Wq.astype(f8).T
        + Wlin.astype(f8).sum(1)[:, None] * bq.astype(f8)[None, :]
        + blin.astype(f8)[:, None]
    )                                                       # [6, msg]
    k = dk.astype(f8) @ Wk.astype(f8).T + bk.astype(f8)[None, :]
    kmod = np.clip(k * k + 2.0 * k + ctx * (1.0 + np.abs(k)), 0.0, 6.0)
    bias_row = kmod @ bq.astype(f8)                         # [6]

    cvt = round_f32r if DTYPE_NAME == "float32r" else (lambda a: np.ascontiguousarray(a, dtype=NP_DT))
    dqT = cvt(dq.T)
    kmod32 = kmod.astype(np.float32)

    js = Wq.shape[0] // n_cores
    in_maps = []
    for s in range(n_cores):
        Wq_s = Wq[s * js:(s + 1) * js, :]                  # [js, din]
        # wqT[p, o, j] = Wq_s[j, o*128+p]
        wq_pre = cvt(
            Wq_s.reshape(js, -1, P).transpose(2, 1, 0)
        )                                                  # [128, din/128, js]
        km_s = kmod32[:, s * js:(s + 1) * js]              # [6, js]
        # kmT[p, jc, c] = km_s[c, jc*128+p]
        km_pre = cvt(
            km_s.T.reshape(-1, P, 6).transpose(1, 0, 2)
        )                                                  # [128, js/128, 6]
        in_maps.append({"dqT": dqT, "wqT": wq_pre, "kmT": km_pre})
    return in_maps, bias_row


def host_finish(partials, bias_row):
    dotT = np.zeros_like(partials[0], dtype=np.float64)
    for p in partials:
        dotT += p
    return ((dotT.T + bias_row[None, :]) / 64.0).astype(np.float32)


def kernel(**inputs):
    from concourse.bass_utils import run_bass_kernel_spmd

    if "nc" not in _NC_CACHE:
        _NC_CACHE["nc"] = build_nc()
    nc = _NC_CACHE["nc"]

    in_maps, bias_row = host_prep(inputs)
    res = run_bass_kernel_spmd(nc, in_maps, core_ids=list(range(N_CORES)))
    partials = [r["dotT"] for r in res.results]
    return host_finish(partials, bias_row)
